# revision 66
# baseline (speedup 1.0000x reference)
"""Trainium2 Bass kernel for nn_DiffusionConv (two-direction GCN conv + relu).

out = relu(gcn(x, W_fwd; src->dst) + gcn(x, W_bwd; dst->src))

Algorithm (validated against the jax reference in numpy):
  gcn(x, W; edges) = D^-1/2 (A + I) D^-1/2 x W  with D = indegree+1.
  The weight GEMM commutes with aggregation, so each device aggregates
  per-edge-scaled feature rows over its edges (self-loops included as
  ordinary lanes), applies W per 128-row tile, adds bias, relu.

Device mapping (one SPMD program on 8 cores):
  - x is quantized int8 per-row (q = round(x/s), s = rowmax/127) and stored
    with 256B row stride (128B payload + 128B pad); dma_gather pulls rows
    with elem_size=128B at stride 256B -- HALF the DMA bytes of the fp16
    version, which is the kernel's dominant cost. The full per-edge
    normalization s[g]*dinv[g]*dinv[k] is folded into a per-lane fp16
    weight table, so ONE int8 table serves both conv directions and the
    self-loop terms (appended as ordinary lanes).
  - nodes are permuted into 128-slot tiles; tiles sharded across cores and
    processed in blocks of 5; per (block, stream) one batched dma_gather
    (int16 idxs; each conv's edge stream is split by gather-node half,
    < 32768 vs >= 32768: 4 streams total).
  - DVE dequantizes each gathered block with ONE fused op: fp16 G' =
    int8 G * w_lane (per-lane broadcast multiply doubles as the cast).
  - M[e, r] = (dst_slot[e] == r) built on DVE with one is_equal per
    (block, stream) against an iota matrix; PE matmul agg_T += G'.T @ M
    accumulates in PSUM. M is M_W=32 columns wide: sections are ordered by
    slot bucket (pos stored mod M_W) and matmuls target the matching PSUM
    column bucket. Sections are packed at their exact cross-core-max
    lengths; boundary chunks shared by two sections get one matmul per
    touching section with foreign lanes SENT-masked.
  - per half-block: one Act copy PSUM->SBUF per conv (fp16), per tile 3 PE
    matmuls (agg_f@Wf + agg_b@Wb + ones*bias into one PSUM), one Act
    relu-copy into the fp16 output buffer, one output DMA per block.
"""

import sys

if "/opt/trn_rl_repo" not in sys.path:
    sys.path.insert(0, "/opt/trn_rl_repo")

import numpy as np

P = 128
M_W = 32   # M matrix width (slot bucket size)
NB = P // M_W  # number of slot buckets
HALF = 32768  # int16 index limit for dma_gather
N_CORES = 8
SENT = 100  # sentinel dst-slot for padded/foreign edge lanes (int8, > 31)
BT = 5     # tiles per block (gather/M/out granularity)
ROWB = 256  # int8 table row stride in bytes (payload 128B + 128B pad)


class Schedule:
    pass


def _assign_slots(cnt, n_tiles, T, n_cores, iters=700000, seed=0):
    """Assign tiles to (core, slot) minimizing sum_j sum_s max_8(count)."""
    order = np.argsort(-cnt.sum(0), kind="stable")
    slot_tiles = [list(order[j * n_cores:(j + 1) * n_cores])
                  for j in range(T)]

    c = cnt.T  # [n_tiles, S]

    def slot_cost(tiles):
        return int(c[tiles].max(axis=0).sum())

    costs = [slot_cost(st) for st in slot_tiles]
    rng = np.random.default_rng(seed)
    ra = rng.integers(0, T, iters)
    rb = rng.integers(0, T, iters)
    ri = rng.integers(0, n_cores, iters)
    rj = rng.integers(0, n_cores, iters)
    for a, b, i, j in zip(ra, rb, ri, rj):
        if a == b:
            continue
        sa, sb = slot_tiles[a], slot_tiles[b]
        sa[i], sb[j] = sb[j], sa[i]
        na, nb_ = slot_cost(sa), slot_cost(sb)
        if na + nb_ < costs[a] + costs[b]:
            costs[a], costs[b] = na, nb_
        else:
            sa[i], sb[j] = sb[j], sa[i]

    tile_core = np.empty(n_tiles, dtype=np.int64)
    tile_slot = np.empty(n_tiles, dtype=np.int64)
    for j, st in enumerate(slot_tiles):
        for ci, tl in enumerate(st):
            tile_core[tl] = ci
            tile_slot[tl] = j
    return tile_core, tile_slot


def preprocess(x, edge_index, W_fwd, b_fwd, W_bwd, b_bwd,
               m_np=np.float16, block_tiles=BT, n_cores=N_CORES):
    N, D = x.shape
    assert D == P
    src = edge_index[0].astype(np.int64)
    dst = edge_index[1].astype(np.int64)

    deg_f = np.bincount(dst, minlength=N) + 1.0
    deg_b = np.bincount(src, minlength=N) + 1.0
    dinv_f = (1.0 / np.sqrt(deg_f)).astype(np.float32)
    dinv_b = (1.0 / np.sqrt(deg_b)).astype(np.float32)

    s_x = np.abs(x).max(1) / 127.0
    s_x[s_x == 0] = 1.0
    s_x = s_x.astype(np.float32)
    q8 = np.clip(np.round(x / s_x[:, None]), -127, 127).astype(np.int8)

    # ---- streams: (fwd, bwd) x (lo, hi gather-node half); self-loops
    # appended as ordinary lanes ----
    loops = np.arange(N, dtype=np.int64)
    k_dir = [np.concatenate([dst, loops]), np.concatenate([src, loops])]
    g_dir = [np.concatenate([src, loops]), np.concatenate([dst, loops])]
    w_dir = [s_x[g_dir[0]] * dinv_f[g_dir[0]] * dinv_f[k_dir[0]],
             s_x[g_dir[1]] * dinv_b[g_dir[1]] * dinv_b[k_dir[1]]]

    total = (np.bincount(k_dir[0], minlength=N)
             + np.bincount(k_dir[1], minlength=N))

    # table rows relabeled hottest-first so the high-traffic gather nodes
    # land in the int16-addressable lo half (fewer, smaller hi sections)
    gtot = (np.bincount(g_dir[0], minlength=N)
            + np.bincount(g_dir[1], minlength=N))
    gorder = np.argsort(-gtot, kind="stable")
    relabel = np.empty(N, dtype=np.int64)
    relabel[gorder] = np.arange(N)
    qt = np.zeros((N, ROWB), dtype=np.int8)
    qt[:, :P] = q8[gorder]

    n_tiles = -(-N // P)
    n_tiles = -(-n_tiles // n_cores) * n_cores
    T = n_tiles // n_cores

    # node -> tile: snake round-robin by decreasing total weight
    order = np.argsort(-total, kind="stable")
    tile_of_rank = np.arange(N) % n_tiles
    sweep = np.arange(N) // n_tiles
    odd = (sweep % 2) == 1
    tile_of_rank[odd] = n_tiles - 1 - tile_of_rank[odd]
    node_tile = np.empty(N, dtype=np.int64)
    node_tile[order] = tile_of_rank
    node_pos = np.empty(N, dtype=np.int64)
    by_tile = np.argsort(node_tile, kind="stable")
    sorted_tiles = node_tile[by_tile]
    pos_seq = np.arange(N) - np.searchsorted(sorted_tiles, sorted_tiles)
    pos_seq = (pos_seq % NB) * M_W + pos_seq // NB
    node_pos[by_tile] = pos_seq
    assert node_pos.max() < P

    # 4 streams: s = dir*2 + half(relabeled gather node)
    streams = []
    for d in range(2):
        gr = relabel[g_dir[d]]
        lo = gr < HALF
        for hh, mask in ((0, lo), (1, ~lo)):
            k = k_dir[d][mask]
            g = gr[mask] - hh * HALF
            w = w_dir[d][mask]
            t = node_tile[k]
            bkt = node_pos[k] // M_W
            o = np.lexsort((g, bkt, t))
            streams.append((t[o], g[o], node_pos[k[o]], w[o]))

    cnt2 = np.zeros((4, n_tiles, NB), dtype=np.int64)
    for s, (tt, gg, pp, ww) in enumerate(streams):
        np.add.at(cnt2[s], (tt, pp // M_W), 1)
    tile_core, tile_slot = _assign_slots(
        cnt2.transpose(0, 2, 1).reshape(4 * NB, n_tiles),
        n_tiles, T, n_cores)

    # exact per-section lengths (max across the 8 cores sharing each slot)
    L2 = np.zeros((4, T, NB), dtype=np.int64)
    for s in range(4):
        for h in range(NB):
            per_slot = np.zeros(T, dtype=np.int64)
            np.maximum.at(per_slot, tile_slot, cnt2[s, :, h])
            L2[s, :, h] = per_slot
    # each (tile, bucket, conv) must have at least one lane (self-loops)
    assert ((L2[0] + L2[1]) >= 1).all() and ((L2[2] + L2[3]) >= 1).all()

    # ---- static schedule layout ----
    blocks = [list(range(b, min(b + block_tiles, T)))
              for b in range(0, T, block_tiles)]

    sec_off = np.zeros((4, T, NB), dtype=np.int64)   # lane off within (b,s)
    spans = np.zeros((4, T, NB, 2), dtype=np.int64)  # [s,t,h] -> c_lo, span
    pos_off2 = np.zeros((4, T, NB), dtype=np.int64)
    idx_meta = []  # per block/stream: (icol0, n_idx, ccol0, n_chunks,
    #                                   pb0, sbs)
    ic = 0
    cc = 0
    pc = 0
    for blk in blocks:
        row = []
        for s in range(4):
            pb0 = pc
            off = 0
            for t in blk:
                for h in range(NB):
                    sec_off[s, t, h] = off
                    ln = int(L2[s, t, h])
                    if ln:
                        c_lo = off // P
                        span = -(-(off + ln) // P) - c_lo
                    else:
                        c_lo, span = 0, 0
                    spans[s, t, h] = (c_lo, span)
                    pos_off2[s, t, h] = pc
                    pc += span
                    off += ln
            n_chunks = -(-off // P)
            n_idx = n_chunks * P
            row.append((ic, n_idx, cc, n_chunks, pb0, pc - pb0))
            ic += n_idx // 16
            cc += n_chunks
        idx_meta.append(row)
    idx_cols = max(ic, 1)
    ccols = max(cc, 1)
    pos_cols = max(pc, 1)
    K = L2.sum(-1)
    # widest per-block pos-span (for the [P, M_W, span] iota table)
    sbs_max = 1
    for bi in range(len(blocks)):
        lo = idx_meta[bi][0][4]
        hi = idx_meta[bi][3][4] + idx_meta[bi][3][5]
        sbs_max = max(sbs_max, hi - lo)

    # ---- per-core packing ----
    slot_node = np.full((n_cores, T * P), -1, dtype=np.int64)
    g_slot = tile_slot[node_tile] * P + node_pos
    for core in range(n_cores):
        m = tile_core[node_tile] == core
        slot_node[core, g_slot[m]] = np.arange(N)[m]

    bias_sum = (b_fwd + b_bwd).astype(np.float32)
    wf = np.ascontiguousarray(W_fwd.astype(m_np))
    wb = np.ascontiguousarray(W_bwd.astype(m_np))
    bias_mat = np.ascontiguousarray(
        np.broadcast_to(bias_sum[None, :], (P, P))).astype(np.float32)
    ones_mat = np.zeros((2, P), dtype=np.float32)
    ones_mat[0] = 1.0
    iota_mat = np.ascontiguousarray(
        np.broadcast_to(np.arange(M_W, dtype=np.int8)[None, None, :],
                        (P, 1, M_W)))

    # selection matrix for on-chip 16->128 idx replication (PE matmul)
    sel_mat = np.zeros((16, P), dtype=np.float32)
    sel_mat[np.arange(P) % 16, np.arange(P)] = 1.0

    in_maps = []
    for core in range(n_cores):
        idx_t = np.zeros((16, idx_cols), dtype=np.int16)
        ws_t = np.zeros((P, pos_cols), dtype=np.float32)
        pos_t = np.full((P, pos_cols), SENT, dtype=np.int64)
        for s in range(4):
            tt, gg, pp, ww = streams[s]
            m = tile_core[tt] == core
            t_loc = tile_slot[tt[m]]
            g_loc = gg[m]
            p_loc = pp[m]
            w_loc = ww[m]
            o = np.argsort(t_loc, kind="stable")
            t_loc, g_loc, p_loc, w_loc = (t_loc[o], g_loc[o], p_loc[o],
                                          w_loc[o])
            starts = np.searchsorted(t_loc, np.arange(T))
            ends = np.searchsorted(t_loc, np.arange(T), side="right")
            for bi, blk in enumerate(blocks):
                icol0, n_idx, ccol0, n_chunks, pb0, sbs = idx_meta[bi][s]
                if n_chunks == 0:
                    continue
                flat_i = np.zeros(n_idx, dtype=np.int16)
                for t in blk:
                    s0, e0 = int(starts[t]), int(ends[t])
                    gl, pl, wl = g_loc[s0:e0], p_loc[s0:e0], w_loc[s0:e0]
                    bk = pl // M_W
                    for h in range(NB):
                        msk = bk == h
                        gv = gl[msk]
                        pv = pl[msk] - h * M_W
                        wv = wl[msk]
                        cntc = len(gv)
                        assert cntc <= int(L2[s, t, h])
                        so = int(sec_off[s, t, h])
                        flat_i[so:so + cntc] = gv
                        c_lo, span = (int(spans[s, t, h, 0]),
                                      int(spans[s, t, h, 1]))
                        if span == 0:
                            continue
                        w2 = np.full(span * P, SENT, dtype=np.int64)
                        w3 = np.zeros(span * P, dtype=np.float32)
                        rel = so - c_lo * P
                        w2[rel:rel + cntc] = pv
                        w3[rel:rel + cntc] = wv
                        po = int(pos_off2[s, t, h])
                        pos_t[:, po:po + span] = w2.reshape(span, P).T
                        ws_t[:, po:po + span] = w3.reshape(span, P).T
                idx_t[:, icol0:icol0 + n_idx // 16] = \
                    flat_i.reshape(-1, 16).T
        in_maps.append({
            "qt": qt, "idx": idx_t,
            "ws": np.ascontiguousarray(ws_t.astype(m_np)),
            "pos": np.ascontiguousarray(pos_t.astype(np.int8)),
            "wf": wf, "wb": wb, "bias": bias_mat, "ones": ones_mat,
            "iota": iota_mat, "sel": sel_mat,
        })

    sch = Schedule()
    sch.T, sch.K, sch.blocks = T, K, blocks
    sch.spans, sch.pos_off2 = spans, pos_off2
    sch.idx_meta = idx_meta
    sch.idx_cols, sch.ccols, sch.pos_cols = idx_cols, ccols, pos_cols
    sch.sbs_max = sbs_max
    sch.in_maps = in_maps
    sch.slot_node = slot_node
    sch.N, sch.n_cores = N, n_cores
    sch.m_np = m_np
    return sch


# ---------------------------------------------------------------------------
# device program
# ---------------------------------------------------------------------------

def _dma_gather_raw(gp, out_ap, in_ap, idxs_ap, num_idxs, elem_size,
                    elem_step, queue_num=0):
    """dma_gather (non-transpose) allowing elem_size_bytes < 256 as long as
    the row stride stays a multiple of 256B (verified on HW). Mirrors
    BassGpSimd.dma_gather's plain path minus the 256B elem_size assert."""
    import concourse.mybir as mybir
    from concourse import ap_utils
    from concourse.bass import exact_div

    assert idxs_ap.dtype == mybir.dt.int16
    assert in_ap.dtype == out_ap.dtype
    assert ap_utils.ap_is_contiguous(out_ap.ap[1:])
    assert ap_utils.ap_is_contiguous(idxs_ap.ap[1:])
    assert in_ap.ap[-1][1] == out_ap.ap[-1][1] == elem_size
    assert in_ap.ap[0][0] == elem_step
    stride_bytes = elem_step * mybir.dt.size(in_ap.dtype)
    stride_bytes_256 = exact_div(stride_bytes, 256)
    _in_ap = gp.lower_ap_dma(in_ap, for_custom_bir_dma=True)
    _idxs_ap = gp.lower_ap(idxs_ap)
    _out_ap = gp.lower_ap(out_ap)
    return gp.add_instruction(
        mybir.InstDMAGatherAnt(
            name=gp.bass.get_next_instruction_name(),
            ins=[*_in_ap, _idxs_ap,
                 gp.lower_val_access(gp.to_reg(num_idxs))],
            outs=[_out_ap],
            transpose=False,
            num_idxs=num_idxs,
            elem_size=elem_size,
            stride_bytes_256=stride_bytes_256,
            gen_mode=0,
            single_packet=False,
            queue_num=queue_num,
            sbuf_tokens_per_rank=0,
            sbuf_free_dim_per_rank=0,
            sbuf_free_dim_pad_per_rank=0,
            sbuf_byte_offset=0,
        ))


def build_program(sch, debug=False, dup=1):
    from contextlib import ExitStack
    import concourse.mybir as mybir
    import concourse.tile as tile
    from concourse import bacc

    m_dt = mybir.dt.from_np(np.dtype(sch.m_np))
    f32 = mybir.dt.float32
    i8 = mybir.dt.int8
    i16 = mybir.dt.int16
    T = sch.T
    N = sch.N

    nc = bacc.Bacc("TRN2", target_bir_lowering=False, debug=debug,
                   num_devices=sch.n_cores, num_swdge_queues=4)

    qt = nc.dram_tensor("qt", [N, ROWB], i8, kind="ExternalInput").ap()
    idx_d = nc.dram_tensor("idx", [16, sch.idx_cols], i16,
                           kind="ExternalInput").ap()
    sel_d = nc.dram_tensor("sel", [16, P], f32, kind="ExternalInput").ap()
    ws_d = nc.dram_tensor("ws", [P, sch.pos_cols], m_dt,
                          kind="ExternalInput").ap()
    pos_d = nc.dram_tensor("pos", [P, sch.pos_cols], i8,
                           kind="ExternalInput").ap()
    wf_d = nc.dram_tensor("wf", [P, P], m_dt, kind="ExternalInput").ap()
    wb_d = nc.dram_tensor("wb", [P, P], m_dt, kind="ExternalInput").ap()
    bias_d = nc.dram_tensor("bias", [P, P], f32, kind="ExternalInput").ap()
    ones_d = nc.dram_tensor("ones", [2, P], f32, kind="ExternalInput").ap()
    iota_d = nc.dram_tensor("iota", [P, 1, M_W], i8,
                            kind="ExternalInput").ap()
    out_d = nc.dram_tensor("out", [P, T, P], m_dt, kind="ExternalOutput").ap()

    lim = min(HALF, N)
    tables = [qt[0:lim, 0:P], qt[lim:N, 0:P],
              qt[0:lim, 0:P], qt[lim:N, 0:P]]

    with tile.TileContext(nc) as tc, ExitStack() as ctx:
        const = ctx.enter_context(tc.tile_pool(name="const", bufs=1))

        def load_const(shape, dt, dram_ap, tag):
            t = const.tile(shape, dtype=dt, tag=tag)
            nc.sync.dma_start(out=t[:], in_=dram_ap)
            return t

        wf_sb = load_const([P, P], m_dt, wf_d, "wf")
        wb_sb = load_const([P, P], m_dt, wb_d, "wb")
        bias_sb = load_const([P, P], f32, bias_d, "bias")
        ones_sb = load_const([2, P], f32, ones_d, "ones")
        iota_sb = load_const([P, 1, M_W], i8, iota_d, "iota")
        sel_sb = load_const([16, P], f32, sel_d, "sel")
        # per-block-sliced tables (loaded inside the loop for overlap)
        idx_sb = const.tile([P, sch.idx_cols], dtype=i16, tag="idx")
        ws_sb = const.tile([P, sch.pos_cols], dtype=m_dt, tag="ws")
        pos_sb = const.tile([P, sch.pos_cols], dtype=i8, tag="pos")

        idx16p = ctx.enter_context(tc.tile_pool(name="idx16", bufs=2))
        idxfp = ctx.enter_context(tc.tile_pool(name="idxf", bufs=2))
        idxpp = ctx.enter_context(tc.tile_pool(name="idxp", bufs=2,
                                               space="PSUM"))
        gq = [ctx.enter_context(tc.tile_pool(name=f"gq{s}", bufs=2))
              for s in range(4)]
        gf = [ctx.enter_context(tc.tile_pool(name=f"gf{s}", bufs=2))
              for s in range(4)]
        mpool = ctx.enter_context(tc.tile_pool(name="m", bufs=2))
        aggp = ctx.enter_context(tc.tile_pool(name="aggp", bufs=2,
                                              space="PSUM"))
        outp = ctx.enter_context(tc.tile_pool(name="outp", bufs=2,
                                              space="PSUM"))
        sbp = ctx.enter_context(tc.tile_pool(name="sbp", bufs=2))
        obp = ctx.enter_context(tc.tile_pool(name="obp", bufs=2))

        qload = [0]
        for _dup in range(dup):
            for bi, blk in enumerate(sch.blocks):
                bt0 = blk[0]
                ng = len(blk)
                ic_lo = sch.idx_meta[bi][0][0]
                ic_hi = (sch.idx_meta[bi][3][0]
                         + sch.idx_meta[bi][3][1] // 16)
                icw = ic_hi - ic_lo
                idx16_sb = idx16p.tile([16, icw], dtype=i16, tag="i16")
                nc.sync.dma_start(out=idx16_sb[:],
                                  in_=idx_d[:, ic_lo:ic_hi])
                pb_lo = sch.idx_meta[bi][0][4]
                pb_hi = sch.idx_meta[bi][3][4] + sch.idx_meta[bi][3][5]
                nc.sync.dma_start(out=pos_sb[:, pb_lo:pb_hi],
                                  in_=pos_d[:, pb_lo:pb_hi])
                nc.sync.dma_start(out=ws_sb[:, pb_lo:pb_hi],
                                  in_=ws_d[:, pb_lo:pb_hi])
                # replicate idx 16 -> 128 partitions on-chip: cast to f32,
                # PE selection matmul, cast back to int16
                for c0 in range(0, icw, 512):
                    cw = min(512, icw - c0)
                    ixf = idxfp.tile([16, 512], dtype=f32, tag="ixf")
                    nc.vector.tensor_copy(out=ixf[:, 0:cw],
                                          in_=idx16_sb[:, c0:c0 + cw])
                    ixp = idxpp.tile([P, 512], dtype=f32, tag="ixp")
                    nc.tensor.matmul(out=ixp[:, 0:cw], lhsT=sel_sb[:],
                                     rhs=ixf[:, 0:cw],
                                     start=True, stop=True)
                    nc.vector.tensor_copy(
                        out=idx_sb[:, ic_lo + c0:ic_lo + c0 + cw],
                        in_=ixp[:, 0:cw])
                gtfs, mts = [], []
                for s in range(4):
                    icol0, n_idx, ccol0, n_chunks, pb0, sbs = \
                        sch.idx_meta[bi][s]
                    if n_chunks == 0:
                        gtfs.append(None)
                        mts.append(None)
                        continue
                    gt = gq[s].tile([P, n_chunks, P], dtype=i8, tag=f"g{s}")
                    for c0 in range(0, n_chunks, 48):
                        nsub = min(48, n_chunks - c0)
                        q = (qload[0] % 8) % 4
                        qload[0] += 1
                        _dma_gather_raw(
                            nc.gpsimd,
                            gt[:, c0:c0 + nsub, :],
                            tables[s],
                            idx_sb[:, icol0 + c0 * 8:
                                   icol0 + (c0 + nsub) * 8],
                            num_idxs=nsub * P,
                            elem_size=P,
                            elem_step=ROWB,
                            queue_num=q,
                        )
                    # int8 -> fp16 cast split between DVE and Act (the
                    # per-lane weight is folded into M, so this is a pure
                    # cast); both engines run big ops on disjoint slices
                    gtf = gf[s].tile([P, n_chunks, P], dtype=m_dt,
                                     tag=f"f{s}")
                    kd = int(n_chunks * 0.22)
                    if kd:
                        nc.vector.tensor_copy(out=gtf[:, 0:kd, :],
                                              in_=gt[:, 0:kd, :])
                    if kd < n_chunks:
                        nc.scalar.copy(out=gtf[:, kd:n_chunks, :],
                                       in_=gt[:, kd:n_chunks, :])
                    # M' = (dst_slot == iota) * w_lane
                    mr = mpool.tile([P, sbs, M_W], dtype=m_dt, tag="mr")
                    nc.vector.tensor_tensor(
                        out=mr[:],
                        in0=pos_sb[:, pb0:pb0 + sbs,
                                   None].to_broadcast([P, sbs, M_W]),
                        in1=iota_sb[:, 0:1, :].to_broadcast([P, sbs, M_W]),
                        op=mybir.AluOpType.is_equal,
                    )
                    mt = mpool.tile([P, sbs, M_W], dtype=m_dt, tag=f"m{s}")
                    nc.vector.tensor_tensor(
                        out=mt[:], in0=mr[:],
                        in1=ws_sb[:, pb0:pb0 + sbs,
                                  None].to_broadcast([P, sbs, M_W]),
                        op=mybir.AluOpType.mult,
                    )
                    mts.append(mt)
                    gtfs.append(gtf)

                ob = obp.tile([P, ng, P], dtype=m_dt, tag="ob")

                h0 = 0
                while h0 < ng:
                    hn = min(3, ng - h0)
                    half = blk[h0:h0 + hn]
                    aggf = aggp.tile([P, hn, P], dtype=f32, tag="aggf")
                    aggb = aggp.tile([P, hn, P], dtype=f32, tag="aggb")
                    for tl, t in enumerate(half):
                        for conv, agg in ((0, aggf), (1, aggb)):
                            for h in range(NB):
                                parts = []
                                for s in (conv * 2, conv * 2 + 1):
                                    c_lo = int(sch.spans[s, t, h, 0])
                                    span = int(sch.spans[s, t, h, 1])
                                    pb0 = sch.idx_meta[bi][s][4]
                                    moff = (int(sch.pos_off2[s, t, h])
                                            - pb0)
                                    for ci in range(span):
                                        parts.append((s, c_lo + ci,
                                                      moff + ci))
                                assert parts
                                ocols = agg[:, tl, h * M_W:(h + 1) * M_W]
                                for pi, (s, gc, mc) in enumerate(parts):
                                    nc.tensor.matmul(
                                        out=ocols,
                                        lhsT=gtfs[s][:, gc, :],
                                        rhs=mts[s][:, mc, :],
                                        start=(pi == 0),
                                        stop=(pi == len(parts) - 1),
                                    )

                    af = sbp.tile([P, hn, P], dtype=m_dt, tag="af")
                    nc.scalar.copy(out=af[:], in_=aggf[:])
                    ab = sbp.tile([P, hn, P], dtype=m_dt, tag="ab")
                    nc.scalar.copy(out=ab[:], in_=aggb[:])

                    op_t = outp.tile([P, hn, P], dtype=f32, tag="op")
                    for tl in range(hn):
                        nc.tensor.matmul(out=op_t[:, tl, :],
                                         lhsT=af[:, tl, :], rhs=wf_sb[:],
                                         start=True, stop=False)
                        nc.tensor.matmul(out=op_t[:, tl, :],
                                         lhsT=ab[:, tl, :], rhs=wb_sb[:],
                                         start=False, stop=False)
                        nc.tensor.matmul(out=op_t[:, tl, :],
                                         lhsT=ones_sb[0:1, :],
                                         rhs=bias_sb[0:1, :],
                                         start=False, stop=True)
                    nc.scalar.activation(
                        out=ob[:, h0:h0 + hn, :], in_=op_t[:],
                        func=mybir.ActivationFunctionType.Relu)
                    h0 += hn

                nc.sync.dma_start(out=out_d[:, bt0:bt0 + ng, :], in_=ob[:])

    nc.compile()
    return nc


# ---------------------------------------------------------------------------
# entry point
# ---------------------------------------------------------------------------

_CACHE = {}


def run_sch(sch, trace=False, **kw):
    from concourse.bass_utils import run_bass_kernel_spmd
    key = ("prog", sch.T, sch.spans.tobytes(), sch.idx_cols,
           sch.pos_cols, str(np.dtype(sch.m_np)))
    if key not in _CACHE:
        _CACHE.clear()
        _CACHE[key] = build_program(sch)
    nc = _CACHE[key]
    return run_bass_kernel_spmd(
        nc, sch.in_maps, core_ids=list(range(sch.n_cores)), trace=trace, **kw)


def assemble(sch, results):
    out = np.zeros((sch.N, P), dtype=np.float32)
    for core in range(sch.n_cores):
        o = results[core]["out"]  # [128, T, 128]; slot (t, p) at o[p, t]
        rows = o.astype(np.float32).transpose(1, 0, 2).reshape(-1, P)
        sn = sch.slot_node[core]
        m = sn >= 0
        out[sn[m]] = rows[m]
    return out


def kernel(x, edge_index, W_fwd, b_fwd, W_bwd, b_bwd):
    x = np.asarray(x, dtype=np.float32)
    edge_index = np.asarray(edge_index, dtype=np.int32)
    sch = preprocess(
        x, edge_index,
        np.asarray(W_fwd, np.float32), np.asarray(b_fwd, np.float32),
        np.asarray(W_bwd, np.float32), np.asarray(b_bwd, np.float32))
    res = run_sch(sch)
    return assemble(sch, res.results)
